# revision 53
# baseline (speedup 1.0000x reference)
"""AIR-GCNII layer (gather + segment-sum + gated residual + identity map)
on 8 Trainium2 NeuronCores.

Strategy: destination-node data parallelism. Nodes are sharded 8 ways by id;
each core owns the edges pointing into its shard. The full (bf16) feature
table — pre-scaled by the per-source degree norm — is replicated into every
core's DRAM so the per-edge source-row gather is core-local. The sparse
aggregation is computed as a sequence of 128-edge chunks grouped by blocks
of GROUP_BLOCKS destinations: dma_gather calls of 8 chunks (1024 edges —
the largest call whose per-engine single_packet descriptor stream fits one
16KB SDMA packet) stream the source rows as random HBM reads, round-robin
across 4 SWDGE queues. Call issue is the pace-setter (the Q7 cluster
serializes descriptor generation at ~2us/call), so the schedule minimizes
call count: call windows span whole (group, src-half) ranges and pad with
row 0 instead of splitting per dst block to trim. A binary fp8 one-hot
scatter matrix (streamed from HBM; on-chip DVE is_equal generation is
supported but measured slower end-to-end) selects the local destination
column, and TensorE accumulates  g.T @ onehot  into a PSUM tile holding the
dst-block's aggregation in transposed ([feat, node]) layout. The
per-destination norm is applied by the PSUM->SBUF move. The gate/mix/GCNII
epilogue runs per 128-node block in the same transposed layout so all weight
matmuls have the contraction dim on partitions.

Host-side work is limited to index preprocessing (grouping/padding edge lists
by destination block, degree counting -> per-node norms, standard GCN graph
preprocessing) and layout/dtype staging of inputs.
"""

import os
import numpy as np
import ml_dtypes

BF16 = ml_dtypes.bfloat16
FP8 = ml_dtypes.float8_e4m3

NCORES = 8
BLK = 128
LO = 32768          # int16 gather-index limit: split sources into two halves
# dst blocks whose chunks share one round of dma_gathers: large groups
# minimize the per-(group, half) ceil slack in the 8-chunk call windows —
# every call costs ~2.3us of cluster-serial Q7 descriptor generation
GROUP_BLOCKS = int(os.environ.get("KERNEL_GROUP_BLOCKS", "4"))
GBUFS = int(os.environ.get("KERNEL_GBUFS", "5"))  # gather/oh pool depth
NQUEUES = int(os.environ.get("KERNEL_NQUEUES", "4"))  # SWDGE queues
DMA_SCRATCH = int(os.environ.get("KERNEL_DMA_SCRATCH", "16384"))
# A single_packet call coalesces each DMA engine's descriptor stream into
# one SDMA packet, and a packet caps at 16KB = 64 x 256B rows: calls above
# 8 chunks (1024 idxs) hang the fabric. Hard HW limit.
CALL_CHUNK_CAP = int(os.environ.get("KERNEL_CALL_CAP", "8"))
# single_packet coalesces each engine's whole descriptor stream into one
# SDMA packet, capping a call at 64 data descs/engine (16KB packet): only
# 8-chunk calls fit. Packet-per-descriptor mode lifts the cap but measured
# ~1.7x slower DMA drain; keep single_packet + 8-chunk calls.
SINGLE_PACKET = os.environ.get("KERNEL_SINGLE_PACKET", "1") == "1"
# Generate the one-hot scatter tiles on-chip (DVE is_equal / ACT square+relu
# against an iota row) instead of streaming 11.4MB of precomputed fp8 one-hot
# from HBM per core.
OH_ONCHIP = os.environ.get("KERNEL_OH_ONCHIP", "0") == "1"

# Filled by kernel() for test.py to inspect.
LAST_RESULT = None


class _Sched:
    pass


def _make_schedule(src, dst, n_nodes):
    """Group edges by (core, dst-block, src-half); pad each group to whole
    128-edge chunks with a schedule identical across cores (SPMD)."""
    E = src.shape[0]
    shard = n_nodes // NCORES
    nblk = (shard + BLK - 1) // BLK
    pad_shard = nblk * BLK

    core = dst // shard
    dl = dst % shard
    blk = dl // BLK
    dloc = (dl % BLK).astype(np.float32)
    half = (src >= LO).astype(np.int64)

    key = (core * nblk + blk) * 2 + half
    order = np.argsort(key, kind="stable")
    cnt = np.bincount(key, minlength=NCORES * nblk * 2).reshape(NCORES, nblk, 2)
    # chunks per (block, half), shared across cores
    nch = (-(-cnt // BLK)).max(axis=0)        # [nblk, 2]
    nch[:, 0] = np.maximum(nch[:, 0], 1)      # >=1 chunk so PSUM gets a start

    s = _Sched()
    s.n_nodes = n_nodes
    s.shard = shard
    s.nblk = nblk
    s.pad_shard = pad_shard

    # chunk layout: per group of GROUP_BLOCKS blocks, all lo chunks of its
    # blocks, then all hi chunks. Assign global chunk indices in that order.
    groups = []
    chunk_of = np.zeros((nblk, 2), dtype=np.int64)  # first chunk id per (b, half)
    cidx = 0
    sizes = []
    rem = nblk
    while rem > GROUP_BLOCKS:
        sizes.append(GROUP_BLOCKS)
        rem -= GROUP_BLOCKS
    # taper the final groups so the un-overlappable tail (last group's
    # matmul + epilogue chain) is as short as possible
    while rem:
        take = max(1, min(rem - 1, (rem + 1) // 2))
        sizes.append(take)
        rem -= take

    starts = np.concatenate([[0], np.cumsum(sizes)]).astype(int)
    for gi in range(len(sizes)):
        bs = list(range(starts[gi], starts[gi + 1]))
        grp = _Sched()
        grp.chunk0 = cidx
        grp.bs = bs
        for b in bs:
            chunk_of[b, 0] = cidx
            cidx += int(nch[b, 0])
        grp.lo_nch = cidx - grp.chunk0
        hi0 = cidx
        for b in bs:
            chunk_of[b, 1] = cidx
            cidx += int(nch[b, 1])
        grp.hi_nch = cidx - hi0
        grp.nch = grp.lo_nch + grp.hi_nch
        groups.append(grp)

    # Emit groups largest-first (tapered small groups last -> short tail).
    groups.sort(key=lambda g: -g.nch)
    # Every slot up to the chunk-padded span is gathered: real edges, then
    # row-0 pads whose one-hot column is all-zero. Trailing -1 trimming
    # would need per-(block, half) call windows, and the extra ~23 calls
    # cost more (~1.5us each of serialized gather issue) than the ~1.5MB
    # of padding they would save.
    M = (nch * BLK).astype(np.int64)            # [nblk, 2]
    for grp in groups:
        grp.blocks = []
        grp.calls = []   # (chunk_start, n_chunks, half, reg_idx_count)
        for b in grp.bs:
            chunks = list(range(chunk_of[b, 0], chunk_of[b, 0] + int(nch[b, 0])))
            chunks += list(range(chunk_of[b, 1], chunk_of[b, 1] + int(nch[b, 1])))
            grp.blocks.append((b, chunks))
        # hi-half calls FIRST: a block's matmul chain needs both its lo and
        # hi chunks, and the lo calls land in block order — so with hi
        # already gathered, each block completes right after its own lo
        # call instead of every block waiting for the group's final call
        for h, c0, n in ((1, grp.chunk0 + grp.lo_nch, grp.hi_nch),
                         (0, grp.chunk0, grp.lo_nch)):
            while n > 0:
                take = min(CALL_CHUNK_CAP, n)
                grp.calls.append((c0, take, h, take * BLK))
                c0 += take
                n -= take
    s.groups = groups
    s.tot_chunks = cidx
    s.tot_idx = cidx * BLK
    s.max_group_chunks = max(g.nch for g in groups)
    s.order = order
    s.cnt = cnt
    s.chunk_of = chunk_of
    s.dloc = dloc
    s.M = M
    return s


def _pack_core_arrays(s, src, core_id):
    """Per-core flat (idx, dloc) arrays in global chunk order.

    Per (block, half) span: real edges, then row-0 pads up to the chunk
    boundary (their one-hot column is all-zero, so they contribute 0)."""
    idx_flat = np.full(s.tot_idx, -1, dtype=np.int16)
    dl_flat = np.full(s.tot_idx, -1.0, dtype=np.float32)

    # edge ranges for this core in s.order: key = (core*nblk + blk)*2 + half
    base = np.concatenate([[0], np.cumsum(s.cnt.reshape(-1))])
    for b in range(s.nblk):
        for h in (0, 1):
            k = (core_id * s.nblk + b) * 2 + h
            e = s.order[base[k]:base[k + 1]]
            n = e.shape[0]
            p0 = int(s.chunk_of[b, h]) * BLK
            idx_flat[p0 + n:p0 + int(s.M[b, h])] = 0
            if n == 0:
                continue
            idx_flat[p0:p0 + n] = (src[e] - (LO if h else 0)).astype(np.int16)
            dl_flat[p0:p0 + n] = s.dloc[e]

    idx_w = np.tile(idx_flat.reshape(-1, 16).T, (8, 1)).copy()       # [128, tot_idx/16]
    if OH_ONCHIP:
        # per-chunk local-dst columns: dlc[e, c] = dloc of slot e in chunk c
        # (pad slots -1 -> the on-chip is_equal never fires -> zero OH row)
        dlc = np.ascontiguousarray(
            dl_flat.reshape(s.tot_chunks, BLK).T.astype(np.float32))
        return idx_w, dlc
    # dense per-chunk binary one-hot scatter tiles: oh[c, e, d] = 1 iff dloc==d
    # (src-norm is folded into the feature table, dst-norm into the epilogue)
    oh = np.zeros((s.tot_chunks, BLK, BLK), dtype=FP8)
    j = np.arange(s.tot_idx)
    valid = dl_flat >= 0
    oh[j[valid] // BLK, j[valid] % BLK, dl_flat[valid].astype(np.int64)] = FP8(1.0)
    ohm = np.ascontiguousarray(oh.transpose(1, 0, 2))
    return idx_w, ohm


def _build_graph(s):
    import concourse.bacc as bacc
    import concourse.mybir as mybir
    from concourse import tile

    bf16 = mybir.dt.bfloat16
    f32 = mybir.dt.float32
    f8 = mybir.dt.float8e4
    i16 = mybir.dt.int16
    AF = mybir.ActivationFunctionType
    OP = mybir.AluOpType

    nc = bacc.Bacc(None, target_bir_lowering=True, debug=False,
                   num_swdge_queues=NQUEUES,
                   dynamic_dma_scratch_size=DMA_SCRATCH)

    feats = nc.dram_tensor("feats", [s.n_nodes, BLK], bf16, kind="ExternalInput")
    x0t = nc.dram_tensor("x0t", [BLK, s.pad_shard], bf16, kind="ExternalInput")
    idx = nc.dram_tensor("idx", [BLK, s.tot_idx // 16], i16, kind="ExternalInput")
    if OH_ONCHIP:
        dlc = nc.dram_tensor("dlc", [BLK, s.tot_chunks], f32,
                             kind="ExternalInput")
        iot = nc.dram_tensor("iot", [BLK, BLK], bf16, kind="ExternalInput")
    else:
        ohm = nc.dram_tensor("ohm", [BLK, s.tot_chunks, BLK], f8,
                             kind="ExternalInput")
    nrm = nc.dram_tensor("nrm", [BLK, s.pad_shard], bf16, kind="ExternalInput")
    w1t = nc.dram_tensor("w1t", [BLK, BLK], bf16, kind="ExternalInput")
    w2t = nc.dram_tensor("w2t", [BLK, BLK], bf16, kind="ExternalInput")
    wlt = nc.dram_tensor("wlt", [BLK, BLK], bf16, kind="ExternalInput")
    b2c = nc.dram_tensor("b2c", [BLK, 1], f32, kind="ExternalInput")
    outT = nc.dram_tensor("outT", [BLK, s.pad_shard], bf16, kind="ExternalOutput")

    with tile.TileContext(nc) as tc:
        with (
            tc.tile_pool(name="const", bufs=1) as cpool,
            tc.tile_pool(name="gath", bufs=GBUFS) as gpool,
            tc.tile_pool(name="oh", bufs=GBUFS) as ohpool,
            tc.tile_pool(name="work", bufs=3) as wpool,
            tc.tile_pool(name="psx", bufs=4, space="PSUM") as psx,
            tc.tile_pool(name="psg", bufs=2, space="PSUM") as psg,
            tc.tile_pool(name="psq", bufs=2, space="PSUM") as psq,
        ):
            # warm the gather path before any data lands: the first extended
            # instruction pays a ~6us Q7 IRAM library load, and each queue
            # has one-time init. Dummy 128-idx gathers of row 0 on every
            # queue overlap that cost with the prologue DMAs.
            # Tile assigns SWDGE completion sems round-robin per DMA call over
            # 8 global lanes, and a lane is locked to one queue: queue must be
            # (emission index) % NQUEUES for EVERY dma_gather, warmups included,
            # so lane k always pairs with queue k % NQUEUES.
            ncalls = 0

            idx_t = cpool.tile([BLK, s.tot_idx // 16], i16)
            cols = s.tot_idx // 16
            head = min(512, cols)  # first gathers' slice lands first
            nc.sync.dma_start(idx_t[:, 0:head], idx[:, 0:head])
            if head < cols:
                nc.sync.dma_start(idx_t[:, head:cols], idx[:, head:cols])
            if OH_ONCHIP:
                # per-chunk dst columns + iota row for on-chip one-hot gen
                dlc_t = cpool.tile([BLK, s.tot_chunks], f32)
                nc.sync.dma_start(dlc_t[:], dlc[:])
                iot_t = cpool.tile([BLK, BLK], bf16)
                nc.sync.dma_start(iot_t[:], iot[:])
            # weights/x0/norm ride the sync (SP) ring so the scalar (ACT)
            # ring keeps a lane for the per-group one-hot loads from t=0
            w1t_t = cpool.tile([BLK, BLK], bf16)
            nc.sync.dma_start(w1t_t[:], w1t[:])
            w2t_t = cpool.tile([BLK, BLK], bf16)
            nc.sync.dma_start(w2t_t[:], w2t[:])
            wlt_t = cpool.tile([BLK, BLK], bf16)
            nc.sync.dma_start(wlt_t[:], wlt[:])
            b2_t = cpool.tile([BLK, 1], f32)
            nc.sync.dma_start(b2_t[:], b2c[:])
            x0_t = cpool.tile([BLK, s.pad_shard], bf16)
            nc.sync.dma_start(x0_t[:], x0t[:])
            nrm_t = cpool.tile([BLK, s.pad_shard], bf16)
            nc.sync.dma_start(nrm_t[:], nrm[:])
            wls_t = cpool.tile([BLK, BLK], bf16)
            nc.vector.tensor_scalar_mul(wls_t[:], wlt_t[:], 0.1)

            oh_tiles = {}

            def oh_pieces(g):
                # oh[:, cl, d] = 1 iff dlc[e, chunk0+cl] == d — wide DVE
                # is_equal ops over ~48 chunks via stride-0 APs (iota
                # repeated per chunk, dlc broadcast along d). Per-chunk ops
                # would pay ~400ns instruction overhead each on HW; one
                # giant op would block the in-order DVE queue for ~19us, so
                # the pieces are interleaved between block epilogues.
                grp = s.groups[g]
                oh_t = ohpool.tile([BLK, s.max_group_chunks, BLK], f8)
                oh_tiles[g] = oh_t
                if OH_ONCHIP:
                    step = 16
                    for c0 in range(0, grp.nch, step):
                        n = min(step, grp.nch - c0)
                        iot_b = (iot_t[:].unsqueeze(1)
                                 .broadcast_to((BLK, n, BLK)))
                        dlc_b = (dlc_t[:, grp.chunk0 + c0:grp.chunk0 + c0 + n]
                                 .unsqueeze(2).broadcast_to((BLK, n, BLK)))
                        yield lambda c0=c0, n=n, iot_b=iot_b, dlc_b=dlc_b: \
                            nc.vector.tensor_tensor(
                                oh_t[:, c0:c0 + n, :], iot_b, dlc_b,
                                OP.is_equal)
                else:
                    # alternate HWDGE rings (ACT/SP) so one-hot streaming
                    # doesn't serialize behind a single ring's queue
                    # (splitting into smaller pieces measured ~30us worse:
                    # the extra ring instructions + sem traffic outweigh
                    # any head-of-line smoothing)
                    eng = nc.scalar if g % 2 == 0 else nc.sync
                    yield lambda eng=eng: eng.dma_start(
                        oh_t[:, 0:grp.nch, :],
                        ohm[:, grp.chunk0:grp.chunk0 + grp.nch, :])

            OH_LOOKAHEAD = GBUFS - 1
            for g in range(min(OH_LOOKAHEAD, len(s.groups))):
                for piece in oh_pieces(g):
                    piece()

            for gi, grp in enumerate(s.groups):
                gt = gpool.tile([BLK, s.max_group_chunks, BLK], bf16)
                next_pieces = (list(oh_pieces(gi + OH_LOOKAHEAD))
                               if gi + OH_LOOKAHEAD < len(s.groups) else [])
                oh_t = oh_tiles.pop(gi)

                for c0, take, h, reg in grp.calls:
                    if reg == 0:
                        continue  # fully-trimmed span tail; ancestors cover it
                    cl = c0 - grp.chunk0
                    i0 = c0 * BLK
                    n = take * BLK
                    base_ap = (feats[0:min(LO, s.n_nodes), :] if h == 0
                               else feats[LO:s.n_nodes, :])
                    q = ncalls % NQUEUES
                    ncalls += 1
                    nc.gpsimd.dma_gather(
                        gt[:, cl:cl + take, :],
                        base_ap,
                        idx_t[:, i0 // 16:(i0 + n) // 16],
                        n, reg, BLK,
                        single_packet=SINGLE_PACKET,
                        queue_num=q,
                    )
                npieces = len(next_pieces)
                emitted = 0
                for bi, (b, chunks) in enumerate(grp.blocks):
                    want = -(-npieces * (bi + 1) // len(grp.blocks))
                    while emitted < want:
                        next_pieces[emitted]()
                        emitted += 1
                    X = psx.tile([BLK, BLK], f32)
                    for k, c in enumerate(chunks):
                        cl = c - grp.chunk0
                        nc.tensor.matmul(
                            X[:], gt[:, cl, :],
                            oh_t[:, cl, :],
                            start=(k == 0), stop=(k == len(chunks) - 1),
                        )
                    if os.environ.get("KERNEL_DEBUG_STAGE") == "agg":
                        O = wpool.tile([BLK, BLK], bf16, tag="o")
                        nc.vector.tensor_mul(O[:], X[:],
                                             nrm_t[:, b * BLK:(b + 1) * BLK])
                        nc.sync.dma_start(outT[:, b * BLK:(b + 1) * BLK], O[:])
                        continue
                    x0b = x0_t[:, b * BLK:(b + 1) * BLK]
                    xbf = wpool.tile([BLK, BLK], bf16, tag="xbf")
                    nc.vector.tensor_mul(xbf[:], X[:],
                                         nrm_t[:, b * BLK:(b + 1) * BLK])
                    P2 = psg.tile([BLK, BLK], f32)
                    nc.tensor.matmul(P2[:], w1t_t[:], xbf[:], start=True, stop=False)
                    nc.tensor.matmul(P2[:], w2t_t[:], x0b, start=False, stop=True)
                    G = wpool.tile([BLK, BLK], bf16, tag="gate")
                    nc.scalar.activation(G[:], P2[:], AF.Sigmoid, bias=b2_t[:, 0:1])
                    U = wpool.tile([BLK, BLK], f32, tag="u")
                    nc.vector.tensor_sub(U[:], xbf[:], x0b)
                    V = wpool.tile([BLK, BLK], f32, tag="v")
                    nc.vector.tensor_mul(V[:], G[:], U[:])
                    M = wpool.tile([BLK, BLK], bf16, tag="m")
                    nc.vector.tensor_add(M[:], V[:], x0b)
                    Q = psq.tile([BLK, BLK], f32)
                    nc.tensor.matmul(Q[:], wls_t[:], M[:])
                    O = wpool.tile([BLK, BLK], bf16, tag="o")
                    nc.vector.scalar_tensor_tensor(
                        O[:], M[:], 0.9, Q[:], OP.mult, OP.add,
                    )
                    nc.sync.dma_start(outT[:, b * BLK:(b + 1) * BLK], O[:])

    nc.compile()
    return nc


def _prepare(features, initial_features, src, dst):
    n_nodes = features.shape[0]
    s = _make_schedule(src, dst, n_nodes)

    degs = np.bincount(dst, minlength=n_nodes).astype(np.float32)
    norm = np.maximum(degs, np.float32(1.0)) ** np.float32(-0.5)

    # fold the source-side norm into the replicated feature table
    feats_bf = np.ascontiguousarray((features * norm[:, None]).astype(BF16))

    per_core = []
    for i in range(NCORES):
        idx_w, ohx = _pack_core_arrays(s, src, i)
        x0 = initial_features[i * s.shard:(i + 1) * s.shard].T
        x0p = np.zeros((BLK, s.pad_shard), dtype=BF16)
        x0p[:, :s.shard] = x0.astype(BF16)
        # dst-side norm, replicated across partitions for the epilogue mult
        nrow = np.zeros(s.pad_shard, dtype=np.float32)
        nrow[:s.shard] = norm[i * s.shard:(i + 1) * s.shard]
        nrm = np.ascontiguousarray(
            np.broadcast_to(nrow, (BLK, s.pad_shard)).astype(BF16))
        per_core.append({
            "feats": feats_bf,
            "x0t": x0p,
            "idx": idx_w,
            ("dlc" if OH_ONCHIP else "ohm"): ohx,
            "nrm": nrm,
        })
    return s, per_core


def _weight_maps(W1, W2, b2, Wl):
    maps = {
        "w1t": np.ascontiguousarray(W1.T).astype(BF16),
        "w2t": np.ascontiguousarray(W2.T).astype(BF16),
        "wlt": np.ascontiguousarray(Wl.T).astype(BF16),
        "b2c": np.ascontiguousarray(b2.astype(np.float32).reshape(BLK, 1)),
    }
    if OH_ONCHIP:
        maps["iot"] = np.ascontiguousarray(
            np.broadcast_to(np.arange(BLK, dtype=np.float32), (BLK, BLK))
            .astype(BF16))
    return maps


def kernel(features, initial_features, src, dst, W1, W2, b2, Wl):
    global LAST_RESULT
    from concourse.bass_utils import run_bass_kernel_spmd

    features = np.asarray(features, dtype=np.float32)
    initial_features = np.asarray(initial_features, dtype=np.float32)
    src = np.asarray(src).astype(np.int64)
    dst = np.asarray(dst).astype(np.int64)
    W1 = np.asarray(W1, dtype=np.float32)
    W2 = np.asarray(W2, dtype=np.float32)
    b2 = np.asarray(b2, dtype=np.float32)
    Wl = np.asarray(Wl, dtype=np.float32)

    s, per_core = _prepare(features, initial_features, src, dst)
    wmaps = _weight_maps(W1, W2, b2, Wl)
    in_maps = [dict(m, **wmaps) for m in per_core]

    nc = _build_graph(s)
    trace = bool(int(os.environ.get("KERNEL_TRACE", "0")))
    res = run_bass_kernel_spmd(nc, in_maps, core_ids=list(range(NCORES)),
                               trace=trace)
    LAST_RESULT = res

    parts = [np.asarray(res.results[i]["outT"])[:, :s.shard].T
             for i in range(NCORES)]
    out = np.concatenate(parts, axis=0).astype(np.float32)
    return np.ascontiguousarray(out)



# revision 54
# speedup vs baseline: 1.0388x; 1.0388x over previous
"""AIR-GCNII layer (gather + segment-sum + gated residual + identity map)
on 8 Trainium2 NeuronCores.

Strategy: destination-node data parallelism. Nodes are sharded 8 ways by id;
each core owns the edges pointing into its shard. The full (bf16) feature
table — pre-scaled by the per-source degree norm — is replicated into every
core's DRAM so the per-edge source-row gather is core-local. The sparse
aggregation is computed as a sequence of 128-edge chunks grouped by blocks
of GROUP_BLOCKS destinations: dma_gather calls of 8 chunks (1024 edges —
the largest call whose per-engine single_packet descriptor stream fits one
16KB SDMA packet) stream the source rows as random HBM reads, round-robin
across 4 SWDGE queues. Call issue is the pace-setter (the Q7 cluster
serializes descriptor generation at ~2us/call), so the schedule minimizes
call count: call windows span whole (group, src-half) ranges and pad with
row 0 instead of splitting per dst block to trim. A binary fp8 one-hot
scatter matrix (streamed from HBM; on-chip DVE is_equal generation is
supported but measured slower end-to-end) selects the local destination
column, and TensorE accumulates  g.T @ onehot  into a PSUM tile holding the
dst-block's aggregation in transposed ([feat, node]) layout. The
per-destination norm is applied by the PSUM->SBUF move. The gate/mix/GCNII
epilogue runs per 128-node block in the same transposed layout so all weight
matmuls have the contraction dim on partitions.

Host-side work is limited to index preprocessing (grouping/padding edge lists
by destination block, degree counting -> per-node norms, standard GCN graph
preprocessing) and layout/dtype staging of inputs.
"""

import os
import numpy as np
import ml_dtypes

BF16 = ml_dtypes.bfloat16
FP8 = ml_dtypes.float8_e4m3

NCORES = 8
BLK = 128
LO = 32768          # int16 gather-index limit: split sources into two halves
# dst blocks whose chunks share one round of dma_gathers: large groups
# minimize the per-(group, half) ceil slack in the 8-chunk call windows —
# every call costs ~2.3us of cluster-serial Q7 descriptor generation
GROUP_BLOCKS = int(os.environ.get("KERNEL_GROUP_BLOCKS", "4"))
GBUFS = int(os.environ.get("KERNEL_GBUFS", "6"))   # gather pool depth
OHBUFS = int(os.environ.get("KERNEL_OHBUFS", "3"))  # one-hot pool depth
NQUEUES = int(os.environ.get("KERNEL_NQUEUES", "4"))  # SWDGE queues
DMA_SCRATCH = int(os.environ.get("KERNEL_DMA_SCRATCH", "16384"))
# A single_packet call coalesces each DMA engine's descriptor stream into
# one SDMA packet, and a packet caps at 16KB = 64 x 256B rows: calls above
# 8 chunks (1024 idxs) hang the fabric. Hard HW limit.
CALL_CHUNK_CAP = int(os.environ.get("KERNEL_CALL_CAP", "8"))
# single_packet coalesces each engine's whole descriptor stream into one
# SDMA packet, capping a call at 64 data descs/engine (16KB packet): only
# 8-chunk calls fit. Packet-per-descriptor mode lifts the cap but measured
# ~1.7x slower DMA drain; keep single_packet + 8-chunk calls.
SINGLE_PACKET = os.environ.get("KERNEL_SINGLE_PACKET", "1") == "1"
# Generate the one-hot scatter tiles on-chip (DVE is_equal / ACT square+relu
# against an iota row) instead of streaming 11.4MB of precomputed fp8 one-hot
# from HBM per core.
OH_ONCHIP = os.environ.get("KERNEL_OH_ONCHIP", "0") == "1"

# Filled by kernel() for test.py to inspect.
LAST_RESULT = None


class _Sched:
    pass


def _make_schedule(src, dst, n_nodes):
    """Group edges by (core, dst-block, src-half); pad each group to whole
    128-edge chunks with a schedule identical across cores (SPMD)."""
    E = src.shape[0]
    shard = n_nodes // NCORES
    nblk = (shard + BLK - 1) // BLK
    pad_shard = nblk * BLK

    core = dst // shard
    dl = dst % shard
    blk = dl // BLK
    dloc = (dl % BLK).astype(np.float32)
    half = (src >= LO).astype(np.int64)

    key = (core * nblk + blk) * 2 + half
    order = np.argsort(key, kind="stable")
    cnt = np.bincount(key, minlength=NCORES * nblk * 2).reshape(NCORES, nblk, 2)
    # chunks per (block, half), shared across cores
    nch = (-(-cnt // BLK)).max(axis=0)        # [nblk, 2]
    nch[:, 0] = np.maximum(nch[:, 0], 1)      # >=1 chunk so PSUM gets a start

    s = _Sched()
    s.n_nodes = n_nodes
    s.shard = shard
    s.nblk = nblk
    s.pad_shard = pad_shard

    # chunk layout: per group of GROUP_BLOCKS blocks, all lo chunks of its
    # blocks, then all hi chunks. Assign global chunk indices in that order.
    groups = []
    chunk_of = np.zeros((nblk, 2), dtype=np.int64)  # first chunk id per (b, half)
    cidx = 0
    sizes = []
    rem = nblk
    while rem > GROUP_BLOCKS:
        sizes.append(GROUP_BLOCKS)
        rem -= GROUP_BLOCKS
    # taper the final groups so the un-overlappable tail (last group's
    # matmul + epilogue chain) is as short as possible
    while rem:
        take = max(1, min(rem - 1, (rem + 1) // 2))
        sizes.append(take)
        rem -= take

    starts = np.concatenate([[0], np.cumsum(sizes)]).astype(int)
    for gi in range(len(sizes)):
        bs = list(range(starts[gi], starts[gi + 1]))
        grp = _Sched()
        grp.chunk0 = cidx
        grp.bs = bs
        for b in bs:
            chunk_of[b, 0] = cidx
            cidx += int(nch[b, 0])
        grp.lo_nch = cidx - grp.chunk0
        hi0 = cidx
        for b in bs:
            chunk_of[b, 1] = cidx
            cidx += int(nch[b, 1])
        grp.hi_nch = cidx - hi0
        grp.nch = grp.lo_nch + grp.hi_nch
        groups.append(grp)

    # Emit groups largest-first (tapered small groups last -> short tail).
    groups.sort(key=lambda g: -g.nch)
    # Every slot up to the chunk-padded span is gathered: real edges, then
    # row-0 pads whose one-hot column is all-zero. Trailing -1 trimming
    # would need per-(block, half) call windows, and the extra ~23 calls
    # cost more (~1.5us each of serialized gather issue) than the ~1.5MB
    # of padding they would save.
    M = (nch * BLK).astype(np.int64)            # [nblk, 2]
    for grp in groups:
        grp.blocks = []
        grp.calls = []   # (chunk_start, n_chunks, half, reg_idx_count)
        for b in grp.bs:
            chunks = list(range(chunk_of[b, 0], chunk_of[b, 0] + int(nch[b, 0])))
            chunks += list(range(chunk_of[b, 1], chunk_of[b, 1] + int(nch[b, 1])))
            grp.blocks.append((b, chunks))
        # hi-half calls FIRST: a block's matmul chain needs both its lo and
        # hi chunks, and the lo calls land in block order — so with hi
        # already gathered, each block completes right after its own lo
        # call instead of every block waiting for the group's final call
        for h, c0, n in ((1, grp.chunk0 + grp.lo_nch, grp.hi_nch),
                         (0, grp.chunk0, grp.lo_nch)):
            while n > 0:
                take = min(CALL_CHUNK_CAP, n)
                grp.calls.append((c0, take, h, take * BLK))
                c0 += take
                n -= take
    s.groups = groups
    s.tot_chunks = cidx
    s.tot_idx = cidx * BLK
    s.max_group_chunks = max(g.nch for g in groups)
    s.order = order
    s.cnt = cnt
    s.chunk_of = chunk_of
    s.dloc = dloc
    s.M = M
    return s


def _pack_core_arrays(s, src, core_id):
    """Per-core flat (idx, dloc) arrays in global chunk order.

    Per (block, half) span: real edges, then row-0 pads up to the chunk
    boundary (their one-hot column is all-zero, so they contribute 0)."""
    idx_flat = np.full(s.tot_idx, -1, dtype=np.int16)
    dl_flat = np.full(s.tot_idx, -1.0, dtype=np.float32)

    # edge ranges for this core in s.order: key = (core*nblk + blk)*2 + half
    base = np.concatenate([[0], np.cumsum(s.cnt.reshape(-1))])
    for b in range(s.nblk):
        for h in (0, 1):
            k = (core_id * s.nblk + b) * 2 + h
            e = s.order[base[k]:base[k + 1]]
            n = e.shape[0]
            p0 = int(s.chunk_of[b, h]) * BLK
            idx_flat[p0 + n:p0 + int(s.M[b, h])] = 0
            if n == 0:
                continue
            idx_flat[p0:p0 + n] = (src[e] - (LO if h else 0)).astype(np.int16)
            dl_flat[p0:p0 + n] = s.dloc[e]

    idx_w = np.tile(idx_flat.reshape(-1, 16).T, (8, 1)).copy()       # [128, tot_idx/16]
    if OH_ONCHIP:
        # per-chunk local-dst columns: dlc[e, c] = dloc of slot e in chunk c
        # (pad slots -1 -> the on-chip is_equal never fires -> zero OH row)
        dlc = np.ascontiguousarray(
            dl_flat.reshape(s.tot_chunks, BLK).T.astype(np.float32))
        return idx_w, dlc
    # dense per-chunk binary one-hot scatter tiles: oh[c, e, d] = 1 iff dloc==d
    # (src-norm is folded into the feature table, dst-norm into the epilogue)
    oh = np.zeros((s.tot_chunks, BLK, BLK), dtype=FP8)
    j = np.arange(s.tot_idx)
    valid = dl_flat >= 0
    oh[j[valid] // BLK, j[valid] % BLK, dl_flat[valid].astype(np.int64)] = FP8(1.0)
    ohm = np.ascontiguousarray(oh.transpose(1, 0, 2))
    return idx_w, ohm


def _build_graph(s):
    import concourse.bacc as bacc
    import concourse.mybir as mybir
    from concourse import tile

    bf16 = mybir.dt.bfloat16
    f32 = mybir.dt.float32
    f8 = mybir.dt.float8e4
    i16 = mybir.dt.int16
    AF = mybir.ActivationFunctionType
    OP = mybir.AluOpType

    nc = bacc.Bacc(None, target_bir_lowering=True, debug=False,
                   num_swdge_queues=NQUEUES,
                   dynamic_dma_scratch_size=DMA_SCRATCH)

    feats = nc.dram_tensor("feats", [s.n_nodes, BLK], bf16, kind="ExternalInput")
    x0t = nc.dram_tensor("x0t", [BLK, s.pad_shard], bf16, kind="ExternalInput")
    idx = nc.dram_tensor("idx", [BLK, s.tot_idx // 16], i16, kind="ExternalInput")
    if OH_ONCHIP:
        dlc = nc.dram_tensor("dlc", [BLK, s.tot_chunks], f32,
                             kind="ExternalInput")
        iot = nc.dram_tensor("iot", [BLK, BLK], bf16, kind="ExternalInput")
    else:
        ohm = nc.dram_tensor("ohm", [BLK, s.tot_chunks, BLK], f8,
                             kind="ExternalInput")
    nrm = nc.dram_tensor("nrm", [BLK, s.pad_shard], bf16, kind="ExternalInput")
    w1t = nc.dram_tensor("w1t", [BLK, BLK], bf16, kind="ExternalInput")
    w2t = nc.dram_tensor("w2t", [BLK, BLK], bf16, kind="ExternalInput")
    wlt = nc.dram_tensor("wlt", [BLK, BLK], bf16, kind="ExternalInput")
    b2c = nc.dram_tensor("b2c", [BLK, 1], f32, kind="ExternalInput")
    outT = nc.dram_tensor("outT", [BLK, s.pad_shard], bf16, kind="ExternalOutput")

    with tile.TileContext(nc) as tc:
        with (
            tc.tile_pool(name="const", bufs=1) as cpool,
            tc.tile_pool(name="gath", bufs=GBUFS) as gpool,
            tc.tile_pool(name="oh", bufs=OHBUFS) as ohpool,
            tc.tile_pool(name="work", bufs=3) as wpool,
            tc.tile_pool(name="psx", bufs=4, space="PSUM") as psx,
            tc.tile_pool(name="psg", bufs=2, space="PSUM") as psg,
            tc.tile_pool(name="psq", bufs=2, space="PSUM") as psq,
        ):
            # warm the gather path before any data lands: the first extended
            # instruction pays a ~6us Q7 IRAM library load, and each queue
            # has one-time init. Dummy 128-idx gathers of row 0 on every
            # queue overlap that cost with the prologue DMAs.
            # Tile assigns SWDGE completion sems round-robin per DMA call over
            # 8 global lanes, and a lane is locked to one queue: queue must be
            # (emission index) % NQUEUES for EVERY dma_gather, warmups included,
            # so lane k always pairs with queue k % NQUEUES.
            ncalls = 0

            idx_t = cpool.tile([BLK, s.tot_idx // 16], i16)
            cols = s.tot_idx // 16
            head = min(512, cols)  # first gathers' slice lands first
            nc.sync.dma_start(idx_t[:, 0:head], idx[:, 0:head])
            if head < cols:
                nc.sync.dma_start(idx_t[:, head:cols], idx[:, head:cols])
            if OH_ONCHIP:
                # per-chunk dst columns + iota row for on-chip one-hot gen
                dlc_t = cpool.tile([BLK, s.tot_chunks], f32)
                nc.sync.dma_start(dlc_t[:], dlc[:])
                iot_t = cpool.tile([BLK, BLK], bf16)
                nc.sync.dma_start(iot_t[:], iot[:])
            # weights/x0/norm ride the sync (SP) ring so the scalar (ACT)
            # ring keeps a lane for the per-group one-hot loads from t=0
            w1t_t = cpool.tile([BLK, BLK], bf16)
            nc.sync.dma_start(w1t_t[:], w1t[:])
            w2t_t = cpool.tile([BLK, BLK], bf16)
            nc.sync.dma_start(w2t_t[:], w2t[:])
            wlt_t = cpool.tile([BLK, BLK], bf16)
            nc.sync.dma_start(wlt_t[:], wlt[:])
            b2_t = cpool.tile([BLK, 1], f32)
            nc.sync.dma_start(b2_t[:], b2c[:])
            x0_t = cpool.tile([BLK, s.pad_shard], bf16)
            nc.sync.dma_start(x0_t[:], x0t[:])
            nrm_t = cpool.tile([BLK, s.pad_shard], bf16)
            nc.sync.dma_start(nrm_t[:], nrm[:])
            wls_t = cpool.tile([BLK, BLK], bf16)
            nc.vector.tensor_scalar_mul(wls_t[:], wlt_t[:], 0.1)

            oh_tiles = {}

            def oh_pieces(g):
                # oh[:, cl, d] = 1 iff dlc[e, chunk0+cl] == d — wide DVE
                # is_equal ops over ~48 chunks via stride-0 APs (iota
                # repeated per chunk, dlc broadcast along d). Per-chunk ops
                # would pay ~400ns instruction overhead each on HW; one
                # giant op would block the in-order DVE queue for ~19us, so
                # the pieces are interleaved between block epilogues.
                grp = s.groups[g]
                oh_t = ohpool.tile([BLK, s.max_group_chunks, BLK], f8)
                oh_tiles[g] = oh_t
                if OH_ONCHIP:
                    step = 16
                    for c0 in range(0, grp.nch, step):
                        n = min(step, grp.nch - c0)
                        iot_b = (iot_t[:].unsqueeze(1)
                                 .broadcast_to((BLK, n, BLK)))
                        dlc_b = (dlc_t[:, grp.chunk0 + c0:grp.chunk0 + c0 + n]
                                 .unsqueeze(2).broadcast_to((BLK, n, BLK)))
                        yield lambda c0=c0, n=n, iot_b=iot_b, dlc_b=dlc_b: \
                            nc.vector.tensor_tensor(
                                oh_t[:, c0:c0 + n, :], iot_b, dlc_b,
                                OP.is_equal)
                else:
                    # alternate HWDGE rings (ACT/SP) so one-hot streaming
                    # doesn't serialize behind a single ring's queue
                    # (splitting into smaller pieces measured ~30us worse:
                    # the extra ring instructions + sem traffic outweigh
                    # any head-of-line smoothing)
                    eng = nc.scalar if g % 2 == 0 else nc.sync
                    yield lambda eng=eng: eng.dma_start(
                        oh_t[:, 0:grp.nch, :],
                        ohm[:, grp.chunk0:grp.chunk0 + grp.nch, :])

            OH_LOOKAHEAD = OHBUFS - 1
            for g in range(min(OH_LOOKAHEAD, len(s.groups))):
                for piece in oh_pieces(g):
                    piece()

            for gi, grp in enumerate(s.groups):
                gt = gpool.tile([BLK, s.max_group_chunks, BLK], bf16)
                next_pieces = (list(oh_pieces(gi + OH_LOOKAHEAD))
                               if gi + OH_LOOKAHEAD < len(s.groups) else [])
                oh_t = oh_tiles.pop(gi)

                for c0, take, h, reg in grp.calls:
                    if reg == 0:
                        continue  # fully-trimmed span tail; ancestors cover it
                    cl = c0 - grp.chunk0
                    i0 = c0 * BLK
                    n = take * BLK
                    base_ap = (feats[0:min(LO, s.n_nodes), :] if h == 0
                               else feats[LO:s.n_nodes, :])
                    q = ncalls % NQUEUES
                    ncalls += 1
                    nc.gpsimd.dma_gather(
                        gt[:, cl:cl + take, :],
                        base_ap,
                        idx_t[:, i0 // 16:(i0 + n) // 16],
                        n, reg, BLK,
                        single_packet=SINGLE_PACKET,
                        queue_num=q,
                    )
                npieces = len(next_pieces)
                emitted = 0
                for bi, (b, chunks) in enumerate(grp.blocks):
                    want = -(-npieces * (bi + 1) // len(grp.blocks))
                    while emitted < want:
                        next_pieces[emitted]()
                        emitted += 1
                    X = psx.tile([BLK, BLK], f32)
                    for k, c in enumerate(chunks):
                        cl = c - grp.chunk0
                        nc.tensor.matmul(
                            X[:], gt[:, cl, :],
                            oh_t[:, cl, :],
                            start=(k == 0), stop=(k == len(chunks) - 1),
                        )
                    if os.environ.get("KERNEL_DEBUG_STAGE") == "agg":
                        O = wpool.tile([BLK, BLK], bf16, tag="o")
                        nc.vector.tensor_mul(O[:], X[:],
                                             nrm_t[:, b * BLK:(b + 1) * BLK])
                        nc.sync.dma_start(outT[:, b * BLK:(b + 1) * BLK], O[:])
                        continue
                    x0b = x0_t[:, b * BLK:(b + 1) * BLK]
                    xbf = wpool.tile([BLK, BLK], bf16, tag="xbf")
                    nc.vector.tensor_mul(xbf[:], X[:],
                                         nrm_t[:, b * BLK:(b + 1) * BLK])
                    P2 = psg.tile([BLK, BLK], f32)
                    nc.tensor.matmul(P2[:], w1t_t[:], xbf[:], start=True, stop=False)
                    nc.tensor.matmul(P2[:], w2t_t[:], x0b, start=False, stop=True)
                    G = wpool.tile([BLK, BLK], bf16, tag="gate")
                    nc.scalar.activation(G[:], P2[:], AF.Sigmoid, bias=b2_t[:, 0:1])
                    U = wpool.tile([BLK, BLK], f32, tag="u")
                    nc.vector.tensor_sub(U[:], xbf[:], x0b)
                    V = wpool.tile([BLK, BLK], f32, tag="v")
                    nc.vector.tensor_mul(V[:], G[:], U[:])
                    M = wpool.tile([BLK, BLK], bf16, tag="m")
                    nc.vector.tensor_add(M[:], V[:], x0b)
                    Q = psq.tile([BLK, BLK], f32)
                    nc.tensor.matmul(Q[:], wls_t[:], M[:])
                    O = wpool.tile([BLK, BLK], bf16, tag="o")
                    nc.vector.scalar_tensor_tensor(
                        O[:], M[:], 0.9, Q[:], OP.mult, OP.add,
                    )
                    nc.sync.dma_start(outT[:, b * BLK:(b + 1) * BLK], O[:])

    nc.compile()
    return nc


def _prepare(features, initial_features, src, dst):
    n_nodes = features.shape[0]
    s = _make_schedule(src, dst, n_nodes)

    degs = np.bincount(dst, minlength=n_nodes).astype(np.float32)
    norm = np.maximum(degs, np.float32(1.0)) ** np.float32(-0.5)

    # fold the source-side norm into the replicated feature table
    feats_bf = np.ascontiguousarray((features * norm[:, None]).astype(BF16))

    per_core = []
    for i in range(NCORES):
        idx_w, ohx = _pack_core_arrays(s, src, i)
        x0 = initial_features[i * s.shard:(i + 1) * s.shard].T
        x0p = np.zeros((BLK, s.pad_shard), dtype=BF16)
        x0p[:, :s.shard] = x0.astype(BF16)
        # dst-side norm, replicated across partitions for the epilogue mult
        nrow = np.zeros(s.pad_shard, dtype=np.float32)
        nrow[:s.shard] = norm[i * s.shard:(i + 1) * s.shard]
        nrm = np.ascontiguousarray(
            np.broadcast_to(nrow, (BLK, s.pad_shard)).astype(BF16))
        per_core.append({
            "feats": feats_bf,
            "x0t": x0p,
            "idx": idx_w,
            ("dlc" if OH_ONCHIP else "ohm"): ohx,
            "nrm": nrm,
        })
    return s, per_core


def _weight_maps(W1, W2, b2, Wl):
    maps = {
        "w1t": np.ascontiguousarray(W1.T).astype(BF16),
        "w2t": np.ascontiguousarray(W2.T).astype(BF16),
        "wlt": np.ascontiguousarray(Wl.T).astype(BF16),
        "b2c": np.ascontiguousarray(b2.astype(np.float32).reshape(BLK, 1)),
    }
    if OH_ONCHIP:
        maps["iot"] = np.ascontiguousarray(
            np.broadcast_to(np.arange(BLK, dtype=np.float32), (BLK, BLK))
            .astype(BF16))
    return maps


def kernel(features, initial_features, src, dst, W1, W2, b2, Wl):
    global LAST_RESULT
    from concourse.bass_utils import run_bass_kernel_spmd

    features = np.asarray(features, dtype=np.float32)
    initial_features = np.asarray(initial_features, dtype=np.float32)
    src = np.asarray(src).astype(np.int64)
    dst = np.asarray(dst).astype(np.int64)
    W1 = np.asarray(W1, dtype=np.float32)
    W2 = np.asarray(W2, dtype=np.float32)
    b2 = np.asarray(b2, dtype=np.float32)
    Wl = np.asarray(Wl, dtype=np.float32)

    s, per_core = _prepare(features, initial_features, src, dst)
    wmaps = _weight_maps(W1, W2, b2, Wl)
    in_maps = [dict(m, **wmaps) for m in per_core]

    nc = _build_graph(s)
    trace = bool(int(os.environ.get("KERNEL_TRACE", "0")))
    res = run_bass_kernel_spmd(nc, in_maps, core_ids=list(range(NCORES)),
                               trace=trace)
    LAST_RESULT = res

    parts = [np.asarray(res.results[i]["outT"])[:, :s.shard].T
             for i in range(NCORES)]
    out = np.concatenate(parts, axis=0).astype(np.float32)
    return np.ascontiguousarray(out)



# revision 55
# speedup vs baseline: 1.0436x; 1.0046x over previous
"""AIR-GCNII layer (gather + segment-sum + gated residual + identity map)
on 8 Trainium2 NeuronCores.

Strategy: destination-node data parallelism. Nodes are sharded 8 ways by id;
each core owns the edges pointing into its shard. The full (bf16) feature
table — pre-scaled by the per-source degree norm — is replicated into every
core's DRAM so the per-edge source-row gather is core-local. The sparse
aggregation is computed as a sequence of 128-edge chunks grouped by blocks
of GROUP_BLOCKS destinations: dma_gather calls of 8 chunks (1024 edges —
the largest call whose per-engine single_packet descriptor stream fits one
16KB SDMA packet) stream the source rows as random HBM reads, round-robin
across 4 SWDGE queues. Call issue is the pace-setter (the Q7 cluster
serializes descriptor generation at ~2us/call), so the schedule minimizes
call count: call windows span whole (group, src-half) ranges and pad with
row 0 instead of splitting per dst block to trim. A binary fp8 one-hot
scatter matrix (streamed from HBM; on-chip DVE is_equal generation is
supported but measured slower end-to-end) selects the local destination
column, and TensorE accumulates  g.T @ onehot  into a PSUM tile holding the
dst-block's aggregation in transposed ([feat, node]) layout. The
per-destination norm is applied by the PSUM->SBUF move. The gate/mix/GCNII
epilogue runs per 128-node block in the same transposed layout so all weight
matmuls have the contraction dim on partitions.

Host-side work is limited to index preprocessing (grouping/padding edge lists
by destination block, degree counting -> per-node norms, standard GCN graph
preprocessing) and layout/dtype staging of inputs.
"""

import os
import numpy as np
import ml_dtypes

BF16 = ml_dtypes.bfloat16
FP8 = ml_dtypes.float8_e4m3

NCORES = 8
BLK = 128
LO = 32768          # int16 gather-index limit: split sources into two halves
# dst blocks whose chunks share one round of dma_gathers: large groups
# minimize the per-(group, half) ceil slack in the 8-chunk call windows —
# every call costs ~2.3us of cluster-serial Q7 descriptor generation
GROUP_BLOCKS = int(os.environ.get("KERNEL_GROUP_BLOCKS", "4"))
GBUFS = int(os.environ.get("KERNEL_GBUFS", "5"))   # gather pool depth
OHBUFS = int(os.environ.get("KERNEL_OHBUFS", "5"))  # one-hot pool depth
NQUEUES = int(os.environ.get("KERNEL_NQUEUES", "4"))  # SWDGE queues
DMA_SCRATCH = int(os.environ.get("KERNEL_DMA_SCRATCH", "16384"))
# A single_packet call coalesces each DMA engine's descriptor stream into
# one SDMA packet, and a packet caps at 16KB = 64 x 256B rows: calls above
# 8 chunks (1024 idxs) hang the fabric. Hard HW limit.
CALL_CHUNK_CAP = int(os.environ.get("KERNEL_CALL_CAP", "8"))
# single_packet coalesces each engine's whole descriptor stream into one
# SDMA packet, capping a call at 64 data descs/engine (16KB packet): only
# 8-chunk calls fit. Packet-per-descriptor mode lifts the cap but measured
# ~1.7x slower DMA drain; keep single_packet + 8-chunk calls.
SINGLE_PACKET = os.environ.get("KERNEL_SINGLE_PACKET", "1") == "1"
# Generate the one-hot scatter tiles on-chip (DVE is_equal / ACT square+relu
# against an iota row) instead of streaming 11.4MB of precomputed fp8 one-hot
# from HBM per core.
OH_ONCHIP = os.environ.get("KERNEL_OH_ONCHIP", "0") == "1"

# Filled by kernel() for test.py to inspect.
LAST_RESULT = None


class _Sched:
    pass


def _make_schedule(src, dst, n_nodes):
    """Group edges by (core, dst-block, src-half); pad each group to whole
    128-edge chunks with a schedule identical across cores (SPMD)."""
    E = src.shape[0]
    shard = n_nodes // NCORES
    nblk = (shard + BLK - 1) // BLK
    pad_shard = nblk * BLK

    core = dst // shard
    dl = dst % shard
    blk = dl // BLK
    dloc = (dl % BLK).astype(np.float32)
    half = (src >= LO).astype(np.int64)

    key = (core * nblk + blk) * 2 + half
    order = np.argsort(key, kind="stable")
    cnt = np.bincount(key, minlength=NCORES * nblk * 2).reshape(NCORES, nblk, 2)
    # chunks per (block, half), shared across cores
    nch = (-(-cnt // BLK)).max(axis=0)        # [nblk, 2]
    nch[:, 0] = np.maximum(nch[:, 0], 1)      # >=1 chunk so PSUM gets a start

    s = _Sched()
    s.n_nodes = n_nodes
    s.shard = shard
    s.nblk = nblk
    s.pad_shard = pad_shard

    # chunk layout: per group of GROUP_BLOCKS blocks, all lo chunks of its
    # blocks, then all hi chunks. Assign global chunk indices in that order.
    groups = []
    chunk_of = np.zeros((nblk, 2), dtype=np.int64)  # first chunk id per (b, half)
    cidx = 0
    sizes = []
    rem = nblk
    while rem > GROUP_BLOCKS:
        sizes.append(GROUP_BLOCKS)
        rem -= GROUP_BLOCKS
    # taper the final groups so the un-overlappable tail (last group's
    # matmul + epilogue chain) is as short as possible
    while rem:
        take = max(1, min(rem - 1, (rem + 1) // 2))
        sizes.append(take)
        rem -= take

    starts = np.concatenate([[0], np.cumsum(sizes)]).astype(int)
    for gi in range(len(sizes)):
        bs = list(range(starts[gi], starts[gi + 1]))
        grp = _Sched()
        grp.chunk0 = cidx
        grp.bs = bs
        for b in bs:
            chunk_of[b, 0] = cidx
            cidx += int(nch[b, 0])
        grp.lo_nch = cidx - grp.chunk0
        hi0 = cidx
        for b in bs:
            chunk_of[b, 1] = cidx
            cidx += int(nch[b, 1])
        grp.hi_nch = cidx - hi0
        grp.nch = grp.lo_nch + grp.hi_nch
        groups.append(grp)

    # Emit groups largest-first (tapered small groups last -> short tail).
    groups.sort(key=lambda g: -g.nch)
    # Every slot up to the chunk-padded span is gathered: real edges, then
    # row-0 pads whose one-hot column is all-zero. Trailing -1 trimming
    # would need per-(block, half) call windows, and the extra ~23 calls
    # cost more (~1.5us each of serialized gather issue) than the ~1.5MB
    # of padding they would save.
    M = (nch * BLK).astype(np.int64)            # [nblk, 2]
    for grp in groups:
        grp.blocks = []
        grp.calls = []   # (chunk_start, n_chunks, half, reg_idx_count)
        for b in grp.bs:
            chunks = list(range(chunk_of[b, 0], chunk_of[b, 0] + int(nch[b, 0])))
            chunks += list(range(chunk_of[b, 1], chunk_of[b, 1] + int(nch[b, 1])))
            grp.blocks.append((b, chunks))
        # hi-half calls FIRST: a block's matmul chain needs both its lo and
        # hi chunks, and the lo calls land in block order — so with hi
        # already gathered, each block completes right after its own lo
        # call instead of every block waiting for the group's final call
        for h, c0, n in ((1, grp.chunk0 + grp.lo_nch, grp.hi_nch),
                         (0, grp.chunk0, grp.lo_nch)):
            while n > 0:
                take = min(CALL_CHUNK_CAP, n)
                grp.calls.append((c0, take, h, take * BLK))
                c0 += take
                n -= take
    s.groups = groups
    s.tot_chunks = cidx
    s.tot_idx = cidx * BLK
    s.max_group_chunks = max(g.nch for g in groups)
    s.order = order
    s.cnt = cnt
    s.chunk_of = chunk_of
    s.dloc = dloc
    s.M = M
    return s


def _pack_core_arrays(s, src, core_id):
    """Per-core flat (idx, dloc) arrays in global chunk order.

    Per (block, half) span: real edges, then row-0 pads up to the chunk
    boundary (their one-hot column is all-zero, so they contribute 0)."""
    idx_flat = np.full(s.tot_idx, -1, dtype=np.int16)
    dl_flat = np.full(s.tot_idx, -1.0, dtype=np.float32)

    # edge ranges for this core in s.order: key = (core*nblk + blk)*2 + half
    base = np.concatenate([[0], np.cumsum(s.cnt.reshape(-1))])
    for b in range(s.nblk):
        for h in (0, 1):
            k = (core_id * s.nblk + b) * 2 + h
            e = s.order[base[k]:base[k + 1]]
            n = e.shape[0]
            p0 = int(s.chunk_of[b, h]) * BLK
            idx_flat[p0 + n:p0 + int(s.M[b, h])] = 0
            if n == 0:
                continue
            idx_flat[p0:p0 + n] = (src[e] - (LO if h else 0)).astype(np.int16)
            dl_flat[p0:p0 + n] = s.dloc[e]

    idx_w = np.tile(idx_flat.reshape(-1, 16).T, (8, 1)).copy()       # [128, tot_idx/16]
    if OH_ONCHIP:
        # per-chunk local-dst columns: dlc[e, c] = dloc of slot e in chunk c
        # (pad slots -1 -> the on-chip is_equal never fires -> zero OH row)
        dlc = np.ascontiguousarray(
            dl_flat.reshape(s.tot_chunks, BLK).T.astype(np.float32))
        return idx_w, dlc
    # dense per-chunk binary one-hot scatter tiles: oh[c, e, d] = 1 iff dloc==d
    # (src-norm is folded into the feature table, dst-norm into the epilogue)
    oh = np.zeros((s.tot_chunks, BLK, BLK), dtype=FP8)
    j = np.arange(s.tot_idx)
    valid = dl_flat >= 0
    oh[j[valid] // BLK, j[valid] % BLK, dl_flat[valid].astype(np.int64)] = FP8(1.0)
    ohm = np.ascontiguousarray(oh.transpose(1, 0, 2))
    return idx_w, ohm


def _build_graph(s):
    import concourse.bacc as bacc
    import concourse.mybir as mybir
    from concourse import tile

    bf16 = mybir.dt.bfloat16
    f32 = mybir.dt.float32
    f8 = mybir.dt.float8e4
    i16 = mybir.dt.int16
    AF = mybir.ActivationFunctionType
    OP = mybir.AluOpType

    nc = bacc.Bacc(None, target_bir_lowering=True, debug=False,
                   num_swdge_queues=NQUEUES,
                   dynamic_dma_scratch_size=DMA_SCRATCH)

    feats = nc.dram_tensor("feats", [s.n_nodes, BLK], bf16, kind="ExternalInput")
    x0t = nc.dram_tensor("x0t", [BLK, s.pad_shard], bf16, kind="ExternalInput")
    idx = nc.dram_tensor("idx", [BLK, s.tot_idx // 16], i16, kind="ExternalInput")
    if OH_ONCHIP:
        dlc = nc.dram_tensor("dlc", [BLK, s.tot_chunks], f32,
                             kind="ExternalInput")
        iot = nc.dram_tensor("iot", [BLK, BLK], bf16, kind="ExternalInput")
    else:
        ohm = nc.dram_tensor("ohm", [BLK, s.tot_chunks, BLK], f8,
                             kind="ExternalInput")
    nrm = nc.dram_tensor("nrm", [BLK, s.pad_shard], bf16, kind="ExternalInput")
    w1t = nc.dram_tensor("w1t", [BLK, BLK], bf16, kind="ExternalInput")
    w2t = nc.dram_tensor("w2t", [BLK, BLK], bf16, kind="ExternalInput")
    wlt = nc.dram_tensor("wlt", [BLK, BLK], bf16, kind="ExternalInput")
    b2c = nc.dram_tensor("b2c", [BLK, 1], f32, kind="ExternalInput")
    outT = nc.dram_tensor("outT", [BLK, s.pad_shard], bf16, kind="ExternalOutput")

    with tile.TileContext(nc) as tc:
        with (
            tc.tile_pool(name="const", bufs=1) as cpool,
            tc.tile_pool(name="gath", bufs=GBUFS) as gpool,
            tc.tile_pool(name="oh", bufs=OHBUFS) as ohpool,
            tc.tile_pool(name="work", bufs=3) as wpool,
            tc.tile_pool(name="psx", bufs=4, space="PSUM") as psx,
            tc.tile_pool(name="psg", bufs=2, space="PSUM") as psg,
            tc.tile_pool(name="psq", bufs=2, space="PSUM") as psq,
        ):
            # warm the gather path before any data lands: the first extended
            # instruction pays a ~6us Q7 IRAM library load, and each queue
            # has one-time init. Dummy 128-idx gathers of row 0 on every
            # queue overlap that cost with the prologue DMAs.
            # Tile assigns SWDGE completion sems round-robin per DMA call over
            # 8 global lanes, and a lane is locked to one queue: queue must be
            # (emission index) % NQUEUES for EVERY dma_gather, warmups included,
            # so lane k always pairs with queue k % NQUEUES.
            ncalls = 0

            idx_t = cpool.tile([BLK, s.tot_idx // 16], i16)
            cols = s.tot_idx // 16
            head = min(512, cols)  # first gathers' slice lands first
            nc.sync.dma_start(idx_t[:, 0:head], idx[:, 0:head])
            if head < cols:
                nc.sync.dma_start(idx_t[:, head:cols], idx[:, head:cols])
            if OH_ONCHIP:
                # per-chunk dst columns + iota row for on-chip one-hot gen
                dlc_t = cpool.tile([BLK, s.tot_chunks], f32)
                nc.sync.dma_start(dlc_t[:], dlc[:])
                iot_t = cpool.tile([BLK, BLK], bf16)
                nc.sync.dma_start(iot_t[:], iot[:])
            # weights/x0/norm ride the sync (SP) ring so the scalar (ACT)
            # ring keeps a lane for the per-group one-hot loads from t=0
            w1t_t = cpool.tile([BLK, BLK], bf16)
            nc.sync.dma_start(w1t_t[:], w1t[:])
            w2t_t = cpool.tile([BLK, BLK], bf16)
            nc.sync.dma_start(w2t_t[:], w2t[:])
            wlt_t = cpool.tile([BLK, BLK], bf16)
            nc.sync.dma_start(wlt_t[:], wlt[:])
            b2_t = cpool.tile([BLK, 1], f32)
            nc.sync.dma_start(b2_t[:], b2c[:])
            x0_t = cpool.tile([BLK, s.pad_shard], bf16)
            nc.sync.dma_start(x0_t[:], x0t[:])
            nrm_t = cpool.tile([BLK, s.pad_shard], bf16)
            nc.sync.dma_start(nrm_t[:], nrm[:])
            wls_t = cpool.tile([BLK, BLK], bf16)
            nc.vector.tensor_scalar_mul(wls_t[:], wlt_t[:], 0.1)

            oh_tiles = {}

            def oh_pieces(g):
                # oh[:, cl, d] = 1 iff dlc[e, chunk0+cl] == d — wide DVE
                # is_equal ops over ~48 chunks via stride-0 APs (iota
                # repeated per chunk, dlc broadcast along d). Per-chunk ops
                # would pay ~400ns instruction overhead each on HW; one
                # giant op would block the in-order DVE queue for ~19us, so
                # the pieces are interleaved between block epilogues.
                grp = s.groups[g]
                oh_t = ohpool.tile([BLK, s.max_group_chunks, BLK], f8)
                oh_tiles[g] = oh_t
                if OH_ONCHIP:
                    step = 16
                    for c0 in range(0, grp.nch, step):
                        n = min(step, grp.nch - c0)
                        iot_b = (iot_t[:].unsqueeze(1)
                                 .broadcast_to((BLK, n, BLK)))
                        dlc_b = (dlc_t[:, grp.chunk0 + c0:grp.chunk0 + c0 + n]
                                 .unsqueeze(2).broadcast_to((BLK, n, BLK)))
                        yield lambda c0=c0, n=n, iot_b=iot_b, dlc_b=dlc_b: \
                            nc.vector.tensor_tensor(
                                oh_t[:, c0:c0 + n, :], iot_b, dlc_b,
                                OP.is_equal)
                else:
                    # alternate HWDGE rings (ACT/SP) so one-hot streaming
                    # doesn't serialize behind a single ring's queue
                    # (splitting into smaller pieces measured ~30us worse:
                    # the extra ring instructions + sem traffic outweigh
                    # any head-of-line smoothing)
                    eng = nc.scalar if g % 2 == 0 else nc.sync
                    yield lambda eng=eng: eng.dma_start(
                        oh_t[:, 0:grp.nch, :],
                        ohm[:, grp.chunk0:grp.chunk0 + grp.nch, :])

            OH_LOOKAHEAD = OHBUFS - 1
            for g in range(min(OH_LOOKAHEAD, len(s.groups))):
                for piece in oh_pieces(g):
                    piece()

            for gi, grp in enumerate(s.groups):
                gt = gpool.tile([BLK, s.max_group_chunks, BLK], bf16)
                next_pieces = (list(oh_pieces(gi + OH_LOOKAHEAD))
                               if gi + OH_LOOKAHEAD < len(s.groups) else [])
                oh_t = oh_tiles.pop(gi)

                for c0, take, h, reg in grp.calls:
                    if reg == 0:
                        continue  # fully-trimmed span tail; ancestors cover it
                    cl = c0 - grp.chunk0
                    i0 = c0 * BLK
                    n = take * BLK
                    base_ap = (feats[0:min(LO, s.n_nodes), :] if h == 0
                               else feats[LO:s.n_nodes, :])
                    q = ncalls % NQUEUES
                    ncalls += 1
                    nc.gpsimd.dma_gather(
                        gt[:, cl:cl + take, :],
                        base_ap,
                        idx_t[:, i0 // 16:(i0 + n) // 16],
                        n, reg, BLK,
                        single_packet=SINGLE_PACKET,
                        queue_num=q,
                    )
                npieces = len(next_pieces)
                emitted = 0
                for bi, (b, chunks) in enumerate(grp.blocks):
                    want = -(-npieces * (bi + 1) // len(grp.blocks))
                    while emitted < want:
                        next_pieces[emitted]()
                        emitted += 1
                    X = psx.tile([BLK, BLK], f32)
                    for k, c in enumerate(chunks):
                        cl = c - grp.chunk0
                        nc.tensor.matmul(
                            X[:], gt[:, cl, :],
                            oh_t[:, cl, :],
                            start=(k == 0), stop=(k == len(chunks) - 1),
                        )
                    if os.environ.get("KERNEL_DEBUG_STAGE") == "agg":
                        O = wpool.tile([BLK, BLK], bf16, tag="o")
                        nc.vector.tensor_mul(O[:], X[:],
                                             nrm_t[:, b * BLK:(b + 1) * BLK])
                        nc.sync.dma_start(outT[:, b * BLK:(b + 1) * BLK], O[:])
                        continue
                    x0b = x0_t[:, b * BLK:(b + 1) * BLK]
                    xbf = wpool.tile([BLK, BLK], bf16, tag="xbf")
                    nc.vector.tensor_mul(xbf[:], X[:],
                                         nrm_t[:, b * BLK:(b + 1) * BLK])
                    P2 = psg.tile([BLK, BLK], f32)
                    nc.tensor.matmul(P2[:], w1t_t[:], xbf[:], start=True, stop=False)
                    nc.tensor.matmul(P2[:], w2t_t[:], x0b, start=False, stop=True)
                    G = wpool.tile([BLK, BLK], bf16, tag="gate")
                    nc.scalar.activation(G[:], P2[:], AF.Sigmoid, bias=b2_t[:, 0:1])
                    U = wpool.tile([BLK, BLK], f32, tag="u")
                    nc.vector.tensor_sub(U[:], xbf[:], x0b)
                    V = wpool.tile([BLK, BLK], f32, tag="v")
                    nc.vector.tensor_mul(V[:], G[:], U[:])
                    M = wpool.tile([BLK, BLK], bf16, tag="m")
                    nc.vector.tensor_add(M[:], V[:], x0b)
                    Q = psq.tile([BLK, BLK], f32)
                    nc.tensor.matmul(Q[:], wls_t[:], M[:])
                    O = wpool.tile([BLK, BLK], bf16, tag="o")
                    nc.vector.scalar_tensor_tensor(
                        O[:], M[:], 0.9, Q[:], OP.mult, OP.add,
                    )
                    nc.sync.dma_start(outT[:, b * BLK:(b + 1) * BLK], O[:])

    nc.compile()
    return nc


def _prepare(features, initial_features, src, dst):
    n_nodes = features.shape[0]
    s = _make_schedule(src, dst, n_nodes)

    degs = np.bincount(dst, minlength=n_nodes).astype(np.float32)
    norm = np.maximum(degs, np.float32(1.0)) ** np.float32(-0.5)

    # fold the source-side norm into the replicated feature table
    feats_bf = np.ascontiguousarray((features * norm[:, None]).astype(BF16))

    per_core = []
    for i in range(NCORES):
        idx_w, ohx = _pack_core_arrays(s, src, i)
        x0 = initial_features[i * s.shard:(i + 1) * s.shard].T
        x0p = np.zeros((BLK, s.pad_shard), dtype=BF16)
        x0p[:, :s.shard] = x0.astype(BF16)
        # dst-side norm, replicated across partitions for the epilogue mult
        nrow = np.zeros(s.pad_shard, dtype=np.float32)
        nrow[:s.shard] = norm[i * s.shard:(i + 1) * s.shard]
        nrm = np.ascontiguousarray(
            np.broadcast_to(nrow, (BLK, s.pad_shard)).astype(BF16))
        per_core.append({
            "feats": feats_bf,
            "x0t": x0p,
            "idx": idx_w,
            ("dlc" if OH_ONCHIP else "ohm"): ohx,
            "nrm": nrm,
        })
    return s, per_core


def _weight_maps(W1, W2, b2, Wl):
    maps = {
        "w1t": np.ascontiguousarray(W1.T).astype(BF16),
        "w2t": np.ascontiguousarray(W2.T).astype(BF16),
        "wlt": np.ascontiguousarray(Wl.T).astype(BF16),
        "b2c": np.ascontiguousarray(b2.astype(np.float32).reshape(BLK, 1)),
    }
    if OH_ONCHIP:
        maps["iot"] = np.ascontiguousarray(
            np.broadcast_to(np.arange(BLK, dtype=np.float32), (BLK, BLK))
            .astype(BF16))
    return maps


def kernel(features, initial_features, src, dst, W1, W2, b2, Wl):
    global LAST_RESULT
    from concourse.bass_utils import run_bass_kernel_spmd

    features = np.asarray(features, dtype=np.float32)
    initial_features = np.asarray(initial_features, dtype=np.float32)
    src = np.asarray(src).astype(np.int64)
    dst = np.asarray(dst).astype(np.int64)
    W1 = np.asarray(W1, dtype=np.float32)
    W2 = np.asarray(W2, dtype=np.float32)
    b2 = np.asarray(b2, dtype=np.float32)
    Wl = np.asarray(Wl, dtype=np.float32)

    s, per_core = _prepare(features, initial_features, src, dst)
    wmaps = _weight_maps(W1, W2, b2, Wl)
    in_maps = [dict(m, **wmaps) for m in per_core]

    nc = _build_graph(s)
    trace = bool(int(os.environ.get("KERNEL_TRACE", "0")))
    res = run_bass_kernel_spmd(nc, in_maps, core_ids=list(range(NCORES)),
                               trace=trace)
    LAST_RESULT = res

    parts = [np.asarray(res.results[i]["outT"])[:, :s.shard].T
             for i in range(NCORES)]
    out = np.concatenate(parts, axis=0).astype(np.float32)
    return np.ascontiguousarray(out)

